# revision 1
# baseline (speedup 1.0000x reference)
"""Linear attention kernel for 8 Trainium2 NeuronCores.

Sharding: core = 2*b + hg  (b in 0..3 batches, hg in 0..1 head-groups of 8 heads).
Fully data-parallel — no collectives; host sums the two head-group partials per
batch. Each core adds bias/2 so the pair-sum carries the full bias.

Per-core math (T=4096 tokens, CH=512 = 8 heads x 64, DIM=1024):
  qT   = (x @ Wq)^T          c-major [CH, T], elu+1
  k,v  = x @ Wk, x @ Wv      token-major [T, CH], elu+1 on k
  kvT  = v^T k  (per head-pair, diagonal 64-blocks valid), accumulated in PSUM
  z    = ones^T k            [1, CH], accumulated in PSUM
  den  = Zblock^T qT         [8, T]   (Zblock = block-diag expansion of z)
  r    = 1/(den + 1e-6); rbc = E^T r  (broadcast r over each head's 64 rows)
  qsc  = qT * rbc
  M    = kvT^T @ W2  (per 128-row ch-tile; off-diag blocks of kvT zeroed)
  y    = qsc^T @ M + bias/2  token-major [T, DIM]
"""

import sys

sys.path.insert(0, "/opt/trn_rl_repo")

import numpy as np

import concourse.bass as bass
import concourse.mybir as mybir
import concourse.tile as tile
from concourse import bacc

F32 = mybir.dt.float32
BF16 = mybir.dt.bfloat16
AF = mybir.ActivationFunctionType

DIM = 1024      # model dim (contraction for projections)
CH = 512        # per-core channels (8 heads x 64)
P = 128

N_CORES = 8
B, T_FULL = 4, 4096


def build_nc(T=T_FULL):
    NTB = T // 512          # 512-token blocks
    nc = bacc.Bacc(None, target_bir_lowering=False, debug=False)

    xT = nc.declare_dram_parameter("xT", [DIM, T], BF16, isOutput=False)
    w1 = nc.declare_dram_parameter("w1", [DIM, 3 * CH], BF16, isOutput=False)
    w2 = nc.declare_dram_parameter("w2", [CH, DIM], BF16, isOutput=False)
    ec = nc.declare_dram_parameter("ec", [8, CH], BF16, isOutput=False)
    y = nc.declare_dram_parameter("y", [T, DIM], F32, isOutput=True)

    with tile.TileContext(nc) as tc:
        with tc.tile_pool(name="persist", bufs=1) as pp:
            # ---- constants / persistent tiles ----
            ones_col = pp.tile([P, 1], BF16, name="ones_col", tag="ones_col")
            nc.vector.memset(ones_col[:, :], 1.0)

            w1t = []
            for ct in range(8):
                t_ = pp.tile([P, 3 * CH], BF16, name=f"w1_{ct}", tag=f"w1_{ct}")
                nc.sync.dma_start(out=t_[:, :], in_=w1[ct * P:(ct + 1) * P, :])
                w1t.append(t_)

            qt = [
                pp.tile([P, T], BF16, name=f"qt_{j}", tag=f"qt_{j}")
                for j in range(4)
            ]

            kvt = [
                pp.tile([P, P], BF16, name=f"kvt_{j}", tag=f"kvt_{j}")
                for j in range(4)
            ]
            zt = pp.tile([1, CH], BF16, name="zt", tag="zt")

            # prefetch phase-B constants early (off the critical path)
            w2t = []
            for j in range(4):
                t_ = pp.tile([P, DIM], BF16, name=f"w2_{j}", tag=f"w2_{j}")
                nc.sync.dma_start(out=t_[:, :], in_=w2[j * P:(j + 1) * P, :])
                w2t.append(t_)
            ec_sb = pp.tile([8, CH], BF16, name="ec_sb", tag="ec_sb")
            nc.sync.dma_start(out=ec_sb[:, :], in_=ec[:, :])

            phase_a(nc, tc, pp, T, NTB, xT, w1t, qt, kvt, zt, ones_col)
            phase_b(nc, tc, pp, T, NTB, w2t, ec_sb, y, qt, kvt, zt)

    nc.compile()
    return nc


def phase_a(nc, tc, pp, T, NTB, xT, w1t, qt, kvt, zt, ones_col):
    with (
        tc.tile_pool(name="phA_sb", bufs=3) as pa,
        tc.tile_pool(name="xload", bufs=16) as xp,
        tc.tile_pool(name="proj_ps", bufs=6, space="PSUM") as proj_ps,
        tc.tile_pool(name="hold_ps", bufs=1, space="PSUM") as hold_ps,
    ):
            # PSUM accumulators held across all of phase A (one bank each).
            # kvps holds 4 interleaved accumulation regions; zero it up
            # front and accumulate with start=False everywhere (hardware
            # bank-clear on start would wipe sibling regions).
            kvps = hold_ps.tile([P, 4 * P], F32, name="kvps", tag="kvps")
            zps = hold_ps.tile([1, CH], F32, name="zps", tag="zps")
            nc.vector.memset(kvps[:, :], 0.0)

            # ---- phase A: projections + kv/z accumulation ----
            for ib in range(NTB):
                tsl = slice(ib * 512, (ib + 1) * 512)
                xt = []
                for ct in range(8):
                    t_ = xp.tile([P, 512], BF16, name=f"xt_{ib}_{ct}", tag="xt")
                    nc.sync.dma_start(out=t_[:, :], in_=xT[ct * P:(ct + 1) * P, tsl])
                    xt.append(t_)

                # q projection (c-major) with elu+1, into persistent qt
                for j in range(4):
                    qps = proj_ps.tile([P, 512], F32, name=f"qps_{ib}_{j}", tag="proj")
                    for ct in range(8):
                        nc.tensor.matmul(
                            qps[:, :],
                            w1t[ct][:, j * P:(j + 1) * P],
                            xt[ct][:, :],
                            start=(ct == 0),
                            stop=(ct == 7),
                        )
                    m_ = pa.tile([P, 512], F32, name=f"qm_{ib}_{j}", tag="elu_m")
                    e_ = pa.tile([P, 512], F32, name=f"qe_{ib}_{j}", tag="elu_e")
                    r_ = pa.tile([P, 512], F32, name=f"qr_{ib}_{j}", tag="elu_r")
                    nc.vector.tensor_scalar_min(m_[:, :], qps[:, :], 0.0)
                    nc.scalar.activation(e_[:, :], m_[:, :], AF.Exp)
                    nc.scalar.activation(r_[:, :], qps[:, :], AF.Relu)
                    nc.vector.tensor_add(qt[j][:, tsl], e_[:, :], r_[:, :])

                # k, v projections (token-major) per 128-token block
                for t in range(4):
                    tok = slice(t * P, (t + 1) * P)
                    kps = proj_ps.tile([P, 512], F32, name=f"kps_{ib}_{t}", tag="proj")
                    for ct in range(8):
                        nc.tensor.matmul(
                            kps[:, :],
                            xt[ct][:, tok],
                            w1t[ct][:, CH:2 * CH],
                            start=(ct == 0),
                            stop=(ct == 7),
                        )
                    km = pa.tile([P, 512], F32, name=f"km_{ib}_{t}", tag="elu_m")
                    ke = pa.tile([P, 512], F32, name=f"ke_{ib}_{t}", tag="elu_e")
                    kr = pa.tile([P, 512], F32, name=f"kr_{ib}_{t}", tag="elu_r")
                    k_sb = pa.tile([P, 512], BF16, name=f"k_{ib}_{t}", tag="k_sb")
                    nc.vector.tensor_scalar_min(km[:, :], kps[:, :], 0.0)
                    nc.scalar.activation(ke[:, :], km[:, :], AF.Exp)
                    nc.scalar.activation(kr[:, :], kps[:, :], AF.Relu)
                    nc.vector.tensor_add(k_sb[:, :], ke[:, :], kr[:, :])

                    vps = proj_ps.tile([P, 512], F32, name=f"vps_{ib}_{t}", tag="proj")
                    for ct in range(8):
                        nc.tensor.matmul(
                            vps[:, :],
                            xt[ct][:, tok],
                            w1t[ct][:, 2 * CH:3 * CH],
                            start=(ct == 0),
                            stop=(ct == 7),
                        )
                    v_sb = pa.tile([P, 512], BF16, name=f"v_{ib}_{t}", tag="v_sb")
                    nc.scalar.copy(v_sb[:, :], vps[:, :])

                    first = (ib == 0 and t == 0)
                    last = (ib == NTB - 1 and t == 3)
                    # z += ones^T k   [1, 512]
                    nc.tensor.matmul(
                        zps[0:1, :], ones_col[:, :], k_sb[:, :],
                        start=first, stop=last, skip_group_check=True,
                    )
                    # kvT[j] += v_pair^T k_pair   [128, 128] per head-pair.
                    # One accumulation group for the whole packed bank:
                    # start clears the bank once, per-element has_written
                    # handles first-write-overwrite for the other pairs.
                    for j in range(4):
                        csl = slice(j * P, (j + 1) * P)
                        nc.tensor.matmul(
                            kvps[:, csl], v_sb[:, csl], k_sb[:, csl],
                            start=False, stop=(last and j == 3),
                            skip_group_check=True,
                        )

            # ---- evict PSUM accumulators before releasing phase-A pools ----
            for j in range(4):
                nc.vector.memset(kvt[j][:, :], 0.0)
                nc.vector.tensor_copy(
                    kvt[j][0:64, 0:64], kvps[0:64, j * P:j * P + 64]
                )
                nc.vector.tensor_copy(
                    kvt[j][64:128, 64:128],
                    kvps[64:128, j * P + 64:(j + 1) * P],
                )
            nc.vector.tensor_copy(zt[0:1, :], zps[0:1, :])


def phase_b(nc, tc, pp, T, NTB, w2t, ec_sb, y, qt, kvt, zt):
            # ---- phase B setup: Zblock, E, Mstack ----
            Zb, Es = [], []
            for j in range(4):
                zb = pp.tile([P, 8], BF16, name=f"Zb_{j}", tag=f"Zb_{j}")
                nc.vector.memset(zb[:, :], 0.0)
                nc.sync.dma_start(
                    out=zb[0:64, 2 * j:2 * j + 1],
                    in_=zt[0:1, j * P:j * P + 64],
                )
                nc.sync.dma_start(
                    out=zb[64:128, 2 * j + 1:2 * j + 2],
                    in_=zt[0:1, j * P + 64:(j + 1) * P],
                )
                Zb.append(zb)
            for j in range(4):
                Es.append(ec_sb[:, j * P:(j + 1) * P])

            with (
                tc.tile_pool(name="phB_sb", bufs=2) as pb,
                tc.tile_pool(name="qsc_pool", bufs=8) as qp,
                tc.tile_pool(name="phB_ps", bufs=2, space="PSUM") as bps,
                tc.tile_pool(name="y_ps", bufs=3, space="PSUM") as yps_pool,
            ):
                Ms = []
                for j in range(4):
                    ms = pp.tile([P, DIM], BF16, name=f"Ms_{j}", tag=f"Ms_{j}")
                    for h in range(2):
                        hsl = slice(h * 512, (h + 1) * 512)
                        mps = bps.tile([P, 512], F32, name=f"mps_{j}_{h}", tag="m", bufs=1)
                        nc.tensor.matmul(
                            mps[:, :], kvt[j][:, :], w2t[j][:, hsl],
                            start=True, stop=True,
                        )
                        nc.vector.tensor_copy(ms[:, hsl], mps[:, :])
                    Ms.append(ms)

                # ---- phase B main, 3 passes to keep the in-order PE fed ----
                rTs = []
                for ib in range(NTB):
                    tsl = slice(ib * 512, (ib + 1) * 512)
                    dps = bps.tile([8, 512], F32, name=f"dps_{ib}", tag="d")
                    for j in range(4):
                        nc.tensor.matmul(
                            dps[:, :], Zb[j][:, :], qt[j][:, tsl],
                            start=(j == 0), stop=(j == 3),
                        )
                    rf = pb.tile([8, 512], F32, name=f"rf_{ib}", tag="rf")
                    nc.vector.tensor_scalar_add(rf[:, :], dps[:, :], 1e-6)
                    rT = pb.tile([8, 512], BF16, name=f"rT_{ib}", tag="rT", bufs=8)
                    with nc.allow_low_precision(reason="r is O(1e-5); bf16 matches op dtype"):
                        nc.vector.reciprocal(rT[:, :], rf[:, :])
                    rTs.append(rT)

                qscs = []
                for ib in range(NTB):
                    tsl = slice(ib * 512, (ib + 1) * 512)
                    qsc = []
                    for j in range(4):
                        bcp = bps.tile([P, 512], F32, name=f"bcp_{ib}_{j}", tag="bc")
                        nc.tensor.matmul(
                            bcp[:, :], Es[j][:, :], rTs[ib][:, :],
                            start=True, stop=True,
                        )
                        qs = qp.tile(
                            [P, 512], BF16, name=f"qsc_{ib}_{j}", tag="qsc", bufs=32
                        )
                        nc.vector.tensor_mul(qs[:, :], qt[j][:, tsl], bcp[:, :])
                        qsc.append(qs)
                    qscs.append(qsc)

                for ib in range(NTB):
                    qsc = qscs[ib]
                    for t in range(4):
                        tok = slice(t * P, (t + 1) * P)
                        row = (ib * 4 + t) * P
                        y_sb = pb.tile(
                            [P, DIM], F32, name=f"y_{ib}_{t}", tag="y_sb", bufs=3
                        )
                        for h in range(2):
                            hsl = slice(h * 512, (h + 1) * 512)
                            yp = yps_pool.tile(
                                [P, 512], F32, name=f"yps_{ib}_{t}_{h}", tag="y"
                            )
                            for j in range(4):
                                nc.tensor.matmul(
                                    yp[:, :], qsc[j][:, tok], Ms[j][:, hsl],
                                    start=(j == 0), stop=(j == 3),
                                )
                            if h == 0:
                                nc.vector.tensor_copy(y_sb[:, hsl], yp[:, :])
                            else:
                                nc.scalar.copy(y_sb[:, hsl], yp[:, :])
                        nc.sync.dma_start(out=y[row:row + P, :], in_=y_sb[:, :])


_NC_CACHE = {}


def _get_nc(T=T_FULL):
    if T not in _NC_CACHE:
        _NC_CACHE[T] = build_nc(T)
    return _NC_CACHE[T]


def make_in_maps(x, W_qkv, W_out, b_out):
    import ml_dtypes

    bf16 = ml_dtypes.bfloat16
    x = np.asarray(x, dtype=np.float32)
    W_qkv = np.asarray(W_qkv, dtype=np.float32).astype(bf16)
    W_out = np.asarray(W_out, dtype=np.float32).astype(bf16)

    xTs = [np.ascontiguousarray(x[b].T.astype(bf16)) for b in range(B)]
    w1s, w2s = [], []
    for hg in range(2):
        cs = slice(hg * CH, (hg + 1) * CH)
        w1s.append(
            np.ascontiguousarray(
                np.concatenate(
                    [W_qkv[:, cs],
                     W_qkv[:, DIM + hg * CH:DIM + (hg + 1) * CH],
                     W_qkv[:, 2 * DIM + hg * CH:2 * DIM + (hg + 1) * CH]],
                    axis=1,
                )
            )
        )
        w2s.append(np.ascontiguousarray(W_out[cs, :]))
    ecm = make_ec().astype(bf16)

    in_maps = []
    for core in range(N_CORES):
        b, hg = core // 2, core % 2
        in_maps.append({"xT": xTs[b], "w1": w1s[hg], "w2": w2s[hg], "ec": ecm})
    return in_maps


def make_ec():
    """E selector: ec[h, j*128+p] = 1 iff head-of-partition-p-in-tile-j == h."""
    ecm = np.zeros((8, CH), dtype=np.float32)
    for j in range(4):
        ecm[2 * j, j * P:j * P + 64] = 1.0
        ecm[2 * j + 1, j * P + 64:(j + 1) * P] = 1.0
    return ecm


def kernel(x, W_qkv, W_out, b_out):
    from concourse.bass_utils import run_bass_kernel_spmd

    nc = _get_nc(T_FULL)
    in_maps = make_in_maps(x, W_qkv, W_out, b_out)
    res = run_bass_kernel_spmd(nc, in_maps, core_ids=list(range(N_CORES))).results
    bo = np.asarray(b_out, dtype=np.float32)
    out = np.empty((B, T_FULL, DIM), dtype=np.float32)
    for b in range(B):
        out[b] = res[2 * b]["y"] + res[2 * b + 1]["y"] + bo
    return out



# revision 2
# speedup vs baseline: 1.1338x; 1.1338x over previous
"""Linear attention kernel for 8 Trainium2 NeuronCores — fp8 DoubleRow version.

Sharding: core = 2*b + hg  (b in 0..3 batches, hg in 0..1 head-groups of 8 heads).
Fully data-parallel — no collectives; host sums the two head-group partials per
batch plus bias.

fp8 (float8_e4m3: max finite 240) is used where error cancels or is verified
negligible: q/k projections (DoubleRow, K=256/instr), qs and Ms for the y
matmul (DoubleRow). v projection, kv, z, den stay bf16 — v's error propagates
linearly to the output and dominates the budget otherwise.

Scales (powers of 2):
  x8    = fp8(16 x)          |x|<=5.42 -> <=87
  Wq8/Wk8 = fp8(1024 W)      |W|<=1/32 -> <=32     proj psum = 2^14 (x @ W)
  qt    = bf16(16 (elu+1))
  k_sb  = bf16(elu+1), v_sb = bf16(x @ Wv)
  dps   = 2^4 q.z ; rf = 2^-4 dps + 1e-6 ; rT = bf16(1/rf)
  bcp   = Es(2^14) @ rT = 2^14 r
  qs    = fp8(qt * bcp) = fp8(2^18 q r)  <= 2^18/min_z ~ 65
  Ms8   = fp8(2 * kvt @ W2)              maxM ~ 55 -> ~110
  y     = psum(qs @ Ms8) * 2^-19 -> bf16

elu identity (3 ops instead of 4):
  s*(elu(t)+1) = min(exp(t + ln s), s) + relu(s*t)
ACT exp (never overflows: |t|<=~6), ACT relu, then fused
scalar_tensor_tensor((e min s) add r) on DVE. GPSIMD is never used for bulk
elementwise work: it is ~10x slower than DVE/ACT and triggers heavy
power-throttling of the whole core.
"""

import sys

sys.path.insert(0, "/opt/trn_rl_repo")

import numpy as np

import concourse.bass as bass
import concourse.mybir as mybir
import concourse.tile as tile
from concourse import bacc

F32 = mybir.dt.float32
BF16 = mybir.dt.bfloat16
FP8 = mybir.dt.float8e4
AF = mybir.ActivationFunctionType
ALU = mybir.AluOpType
DR = mybir.MatmulPerfMode.DoubleRow

DIM = 1024      # model dim (contraction for projections)
CH = 512        # per-core channels (8 heads x 64)
P = 128

N_CORES = 8
B, T_FULL = 4, 4096

LN16 = float(np.log(16.0))
MS_SCALE = 2.0


def build_nc(T=T_FULL):
    NTB = T // 512          # 512-token blocks
    nc = bacc.Bacc(None, target_bir_lowering=False, debug=False)

    # xT c-major fp8(16x); xV c-major bf16 (v path needs full precision).
    # w1 fp8 [DIM, 2CH]: q | k (1024-scaled). wv bf16 [DIM, CH].
    xT = nc.declare_dram_parameter("xT", [DIM, T], FP8, isOutput=False)
    xV = nc.declare_dram_parameter("xV", [DIM, T], BF16, isOutput=False)
    w1 = nc.declare_dram_parameter("w1", [DIM, 2 * CH], FP8, isOutput=False)
    wv = nc.declare_dram_parameter("wv", [DIM, CH], BF16, isOutput=False)
    w2 = nc.declare_dram_parameter("w2", [CH, DIM], BF16, isOutput=False)
    ec = nc.declare_dram_parameter("ec", [P, CH], BF16, isOutput=False)
    y = nc.declare_dram_parameter("y", [T, DIM], BF16, isOutput=True)

    with tile.TileContext(nc) as tc:
        with tc.tile_pool(name="persist", bufs=1) as pp:
            # ---- constants / persistent tiles ----
            ones_col = pp.tile([P, 1], BF16, name="ones_col", tag="ones_col")
            nc.vector.memset(ones_col[:, :], 1.0)
            bias_q = pp.tile([P, 1], F32, name="bias_q", tag="bias_q")
            nc.vector.memset(bias_q[:, :], LN16)

            # w1 tiles: [P, 2, 2CH] fp8, ct2 = pair of 128-row k-blocks.
            # Loaded first: the single usable DMA queue (sync) serializes all
            # transfers, so only what the first q matmuls need goes ahead of
            # the ib0 x tiles; wv/w2/ec stream in behind them (phase_a).
            w1t = []
            for ct2 in range(4):
                t_ = pp.tile([P, 2, 2 * CH], FP8, name=f"w1_{ct2}", tag=f"w1_{ct2}")
                for i in range(2):
                    r0 = ct2 * 256 + i * 128
                    nc.sync.dma_start(out=t_[:, i, :], in_=w1[r0:r0 + P, :])
                w1t.append(t_)
            wvts = []
            for ct in range(8):
                t_ = pp.tile([P, CH], BF16, name=f"wv_{ct}", tag=f"wv_{ct}")
                wvts.append(t_)

            # qt: bf16 c-major q, jp-paired layout [P, 2, T]
            qt = [
                pp.tile([P, 2, T], BF16, name=f"qt_{jp}", tag=f"qt_{jp}")
                for jp in range(2)
            ]

            kvt = [
                pp.tile([P, P], BF16, name=f"kvt_{j}", tag=f"kvt_{j}")
                for j in range(4)
            ]
            zt = pp.tile([1, CH], BF16, name="zt", tag="zt")

            w2t = []
            for j in range(4):
                t_ = pp.tile([P, DIM], BF16, name=f"w2_{j}", tag=f"w2_{j}")
                w2t.append(t_)
            ec_sb = pp.tile([P, CH], BF16, name="ec_sb", tag="ec_sb")

            rTbanks = [
                pp.tile([P, 512], BF16, name=f"rT_{g4}", tag=f"rT_{g4}")
                for g4 in range(NTB // 4)
            ]
            Zb = []
            for j in range(4):
                zb = pp.tile([P, 8], BF16, name=f"Zb_{j}", tag=f"Zb_{j}")
                Zb.append(zb)

            phase_a(nc, tc, pp, T, NTB, xT, xV, w1t, wvts, qt, kvt, zt,
                    ones_col, bias_q, wv, w2, (ec, ec_sb), w2t, Zb, rTbanks)
            phase_b(nc, tc, pp, T, NTB, w2t, ec_sb, y, qt, kvt, zt, Zb, rTbanks)

    nc.compile()
    return nc


def emit_q(nc, pa, proj_ps, w1t, xt, qt, bias_q, ib, tsl):
    # q projection (c-major, fp8 DR) with elu+1 -> bf16 qt
    for j in range(4):
        jp, ji = j // 2, j % 2
        qps = proj_ps.tile([P, 512], F32, name=f"qps_{ib}_{j}", tag="proj")
        for ct2 in range(4):
            nc.tensor.matmul(
                qps[:, :],
                w1t[ct2][:, :, j * P:(j + 1) * P],
                xt[ct2][:, :, :],
                start=(ct2 == 0),
                stop=(ct2 == 3),
                perf_mode=DR,
            )
        e_ = pa.tile([P, 512], F32, name=f"qe_{ib}_{j}", tag="elu_e")
        r_ = pa.tile([P, 512], F32, name=f"qr_{ib}_{j}", tag="elu_r")
        nc.scalar.activation(e_[:, :], qps[:, :], AF.Exp,
                             bias=bias_q[:, :], scale=float(2.0 ** -14))
        nc.scalar.activation(r_[:, :], qps[:, :], AF.Relu,
                             scale=float(2.0 ** -10))
        nc.vector.scalar_tensor_tensor(
            out=qt[jp][:, ji, tsl], in0=e_[:, :], scalar=16.0,
            in1=r_[:, :], op0=ALU.min, op1=ALU.add,
        )


def phase_a(nc, tc, pp, T, NTB, xT, xV, w1t, wvts, qt, kvt, zt,
            ones_col, bias_q, wv, w2, ec, w2t, Zb, rTbanks):
    deferred_q = []
    with (
        tc.tile_pool(name="phA_sb", bufs=3) as pa,
        tc.tile_pool(name="kv_sb", bufs=4) as kvp,
        tc.tile_pool(name="xload", bufs=8) as xp,
        tc.tile_pool(name="xvload", bufs=16) as xvp,
        tc.tile_pool(name="proj_ps", bufs=6, space="PSUM") as proj_ps,
        tc.tile_pool(name="hold_ps", bufs=1, space="PSUM") as hold_ps,
    ):
        # PSUM accumulators held across all of phase A (one bank each).
        kvps = hold_ps.tile([P, 4 * P], F32, name="kvps", tag="kvps")
        zps = hold_ps.tile([1, CH], F32, name="zps", tag="zps")
        nc.vector.memset(kvps[:, :], 0.0)

        for ib in range(NTB):
            tsl = slice(ib * 512, (ib + 1) * 512)
            xt, xv = [], []
            for ct2 in range(4):
                t_ = xp.tile([P, 2, 512], FP8, name=f"xt_{ib}_{ct2}", tag="xt")
                for i in range(2):
                    r0 = ct2 * 256 + i * 128
                    nc.sync.dma_start(out=t_[:, i, :], in_=xT[r0:r0 + P, tsl])
                xt.append(t_)
            for ct in range(8):
                t_ = xvp.tile([P, 512], BF16, name=f"xv_{ib}_{ct}", tag="xv")
                nc.gpsimd.dma_start(out=t_[:, :], in_=xV[ct * P:(ct + 1) * P, tsl])
                xv.append(t_)
            if ib == 0:
                # behind ib0's x tiles: v weights (needed in ~5us), then the
                # phase-B constants
                for ct in range(8):
                    nc.sync.dma_start(out=wvts[ct][:, :],
                                      in_=wv[ct * P:(ct + 1) * P, :])
                for j in range(4):
                    nc.sync.dma_start(out=w2t[j][:, :],
                                      in_=w2[j * P:(j + 1) * P, :])
                nc.sync.dma_start(out=ec[1][:, :], in_=ec[0][:, :])

            if ib < NTB - 2:
                emit_q(nc, pa, proj_ps, w1t, xt, qt, bias_q, ib, tsl)
            else:
                deferred_q.append((ib, tsl, xt))

            # ---- k (fp8 DR), v (bf16) per 128-token block ----
            for t in range(4):
                tok = slice(t * P, (t + 1) * P)
                kps = proj_ps.tile([P, 512], F32, name=f"kps_{ib}_{t}", tag="proj")
                for ct2 in range(4):
                    nc.tensor.matmul(
                        kps[:, :],
                        xt[ct2][:, :, tok],
                        w1t[ct2][:, :, CH:2 * CH],
                        start=(ct2 == 0),
                        stop=(ct2 == 3),
                        perf_mode=DR,
                    )
                ke = pa.tile([P, 512], F32, name=f"ke_{ib}_{t}", tag="elu_e")
                kr = pa.tile([P, 512], F32, name=f"kr_{ib}_{t}", tag="elu_r")
                k_sb = kvp.tile([P, 512], BF16, name=f"k_{ib}_{t}", tag="k_sb")
                nc.scalar.activation(ke[:, :], kps[:, :], AF.Exp,
                                     bias=0.0, scale=float(2.0 ** -14))
                nc.vector.tensor_scalar(
                    out=kr[:, :], in0=kps[:, :],
                    scalar1=float(2.0 ** -14), scalar2=0.0,
                    op0=ALU.mult, op1=ALU.max,
                )
                nc.vector.scalar_tensor_tensor(
                    out=k_sb[:, :], in0=ke[:, :], scalar=1.0,
                    in1=kr[:, :], op0=ALU.min, op1=ALU.add,
                )

                vps = proj_ps.tile([P, 512], F32, name=f"vps_{ib}_{t}", tag="proj")
                for ct in range(8):
                    nc.tensor.matmul(
                        vps[:, :],
                        xv[ct][:, tok],
                        wvts[ct][:, :],
                        start=(ct == 0),
                        stop=(ct == 7),
                    )
                v_sb = kvp.tile([P, 512], BF16, name=f"v_{ib}_{t}", tag="v_sb")
                nc.scalar.activation(v_sb[:, :], vps[:, :], AF.Copy)

                first = (ib == 0 and t == 0)
                last = (ib == NTB - 1 and t == 3)
                # z += ones^T k   [1, 512]
                nc.tensor.matmul(
                    zps[0:1, :], ones_col[:, :], k_sb[:, :],
                    start=first, stop=last, skip_group_check=True,
                )
                # kvT[j] += v_pair^T k_pair   [128, 128] per head-pair.
                for j in range(4):
                    csl = slice(j * P, (j + 1) * P)
                    nc.tensor.matmul(
                        kvps[:, csl], v_sb[:, csl], k_sb[:, csl],
                        start=False, stop=(last and j == 3),
                        skip_group_check=True,
                    )

        # deferred q projections: PE filler while DVE evicts kv/z and Zb loads
        for ib, tsl, xt in deferred_q:
            emit_q(nc, pa, proj_ps, w1t, xt, qt, bias_q, ib, tsl)

        # ---- evict PSUM accumulators before releasing phase-A pools ----
        for j in range(4):
            nc.vector.memset(kvt[j][:, :], 0.0)
            nc.vector.tensor_copy(
                kvt[j][0:64, 0:64], kvps[0:64, j * P:j * P + 64]
            )
            nc.vector.tensor_copy(
                kvt[j][64:128, 64:128],
                kvps[64:128, j * P + 64:(j + 1) * P],
            )
        nc.vector.tensor_copy(zt[0:1, :], zps[0:1, :])


def phase_b(nc, tc, pp, T, NTB, w2t, ec_sb, y, qt, kvt, zt, Zb, rTbanks):
    for j in range(4):
        nc.vector.memset(Zb[j][:, :], 0.0)
        nc.sync.dma_start(
            out=Zb[j][0:64, 2 * j:2 * j + 1],
            in_=zt[0:1, j * P:j * P + 64],
        )
        nc.sync.dma_start(
            out=Zb[j][64:128, 2 * j + 1:2 * j + 2],
            in_=zt[0:1, j * P + 64:(j + 1) * P],
        )
    # Es copies at partition bases 0/32/64/96: bc for ib uses base 32*(ib%4)
    Es = [[ec_sb[32 * g:32 * g + 8, j * P:(j + 1) * P] for j in range(4)]
          for g in range(4)]

    with (
        tc.tile_pool(name="phB_sb", bufs=2) as pb,
        tc.tile_pool(name="qsc_pool", bufs=4) as qp,
        tc.tile_pool(name="phB_ps", bufs=2, space="PSUM") as bps,
        tc.tile_pool(name="y_ps", bufs=4, space="PSUM") as yps_pool,
    ):
        # den: 4 token-blocks per psum bank at partition bases 0/32/64/96 ->
        # ONE full-width rf + reciprocal serves 4 blocks
        for g4 in range(NTB // 4):
            dps = bps.tile([P, 512], F32, name=f"dps_{g4}", tag="d", bufs=2)
            nc.vector.memset(dps[:, :], 0.0)
            for gi in range(4):
                ib = g4 * 4 + gi
                tsl = slice(ib * 512, (ib + 1) * 512)
                for j in range(4):
                    jp, ji = j // 2, j % 2
                    nc.tensor.matmul(
                        dps[32 * gi:32 * gi + 8, :], Zb[j][:, :],
                        qt[jp][:, ji, tsl],
                        start=False, stop=(j == 3),
                        skip_group_check=True,
                        tile_position=(0, 32 * gi),
                    )
            rf = pb.tile([P, 512], F32, name=f"rf_{g4}", tag="rf")
            nc.vector.tensor_scalar(
                out=rf[:, :], in0=dps[:, :], scalar1=float(2.0 ** -4),
                scalar2=1e-6, op0=ALU.mult, op1=ALU.add,
            )
            with nc.allow_low_precision(reason="r is O(1e-5); bf16 ok"):
                nc.vector.reciprocal(rTbanks[g4][:, :], rf[:, :])

        # Ms: fp8 jp-paired [P, 2, DIM], scaled by MS_SCALE
        Ms = []
        for jp in range(2):
            ms = pp.tile([P, 2, DIM], FP8, name=f"Ms_{jp}", tag=f"Ms_{jp}")
            for i in range(2):
                j = 2 * jp + i
                for h in range(2):
                    hsl = slice(h * 512, (h + 1) * 512)
                    mps = bps.tile([P, 512], F32, name=f"mps_{j}_{h}", tag="d",
                                   bufs=2)
                    nc.tensor.matmul(
                        mps[:, :], kvt[j][:, :], w2t[j][:, hsl],
                        start=True, stop=True,
                    )
                    if h == 0:
                        nc.vector.tensor_scalar_mul(ms[:, i, hsl], mps[:, :],
                                                    MS_SCALE)
                    else:
                        nc.scalar.activation(ms[:, i, hsl], mps[:, :], AF.Copy,
                                             scale=MS_SCALE)
            Ms.append(ms)

        qscs = []
        for ib in range(NTB):
            tsl = slice(ib * 512, (ib + 1) * 512)
            qsc = []
            for jp in range(2):
                qs = qp.tile([P, 2, 512], FP8, name=f"qsc_{ib}_{jp}",
                             tag="qsc", bufs=12)
                for i in range(2):
                    j = 2 * jp + i
                    g4, gi = ib // 4, ib % 4
                    bcp = bps.tile([P, 512], F32, name=f"bcp_{ib}_{j}", tag="bc")
                    nc.tensor.matmul(
                        bcp[:, :], Es[gi][j][:, :],
                        rTbanks[g4][32 * gi:32 * gi + 8, :],
                        start=True, stop=True,
                        tile_position=(32 * gi, 0),
                    )
                    nc.vector.tensor_tensor(
                        out=qs[:, i, :], in0=qt[jp][:, i, tsl], in1=bcp[:, :],
                        op=ALU.mult,
                    )
                qsc.append(qs)
            qscs.append(qsc)

        YS = float(2.0 ** -18 / MS_SCALE)
        for ib in range(NTB):
            qsc = qscs[ib]
            for t in range(4):
                tok = slice(t * P, (t + 1) * P)
                row = (ib * 4 + t) * P
                y_sb = pb.tile(
                    [P, DIM], BF16, name=f"y_{ib}_{t}", tag="y_sb", bufs=3
                )
                for h in range(2):
                    hsl = slice(h * 512, (h + 1) * 512)
                    yp = yps_pool.tile(
                        [P, 512], F32, name=f"yps_{ib}_{t}_{h}", tag="y"
                    )
                    for jp in range(2):
                        nc.tensor.matmul(
                            yp[:, :], qsc[jp][:, :, tok], Ms[jp][:, :, hsl],
                            start=(jp == 0), stop=(jp == 1),
                            perf_mode=DR,
                        )
                    if h == 0:
                        nc.vector.tensor_scalar_mul(y_sb[:, hsl], yp[:, :], YS)
                    else:
                        nc.scalar.activation(y_sb[:, hsl], yp[:, :], AF.Copy,
                                             scale=YS)
                    nc.sync.dma_start(out=y[row:row + P, hsl],
                                      in_=y_sb[:, hsl])


_NC_CACHE = {}


def _get_nc(T=T_FULL):
    if T not in _NC_CACHE:
        _NC_CACHE[T] = build_nc(T)
    return _NC_CACHE[T]


def make_in_maps(x, W_qkv, W_out, b_out):
    import ml_dtypes

    bf16 = ml_dtypes.bfloat16
    f8 = ml_dtypes.float8_e4m3
    x = np.asarray(x, dtype=np.float32)
    W_qkv = np.asarray(W_qkv, dtype=np.float32)
    W_out = np.asarray(W_out, dtype=np.float32).astype(bf16)

    xTs, xVs = [], []
    for b in range(B):
        xt = np.ascontiguousarray(x[b].T)
        xTs.append(np.ascontiguousarray((16.0 * xt).astype(f8)))
        xVs.append(np.ascontiguousarray(xt.astype(bf16)))
    w1s, wvs, w2s = [], [], []
    for hg in range(2):
        cs = slice(hg * CH, (hg + 1) * CH)
        Wq = W_qkv[:, cs]
        Wk = W_qkv[:, DIM + hg * CH:DIM + (hg + 1) * CH]
        Wv = W_qkv[:, 2 * DIM + hg * CH:2 * DIM + (hg + 1) * CH]
        Wq8 = (1024.0 * Wq).astype(f8)
        Wk8 = (1024.0 * Wk).astype(f8)
        w1s.append(np.ascontiguousarray(np.concatenate([Wq8, Wk8], axis=1)))
        wvs.append(np.ascontiguousarray(Wv.astype(bf16)))
        w2s.append(np.ascontiguousarray(W_out[cs, :]))
    ec1 = make_ec() * (2.0 ** 14)
    ecm = np.zeros((P, CH), dtype=np.float32)
    for g in range(4):
        ecm[32 * g:32 * g + 8, :] = ec1
    ecm = ecm.astype(bf16)

    in_maps = []
    for core in range(N_CORES):
        b, hg = core // 2, core % 2
        in_maps.append({"xT": xTs[b], "xV": xVs[b], "w1": w1s[hg],
                        "wv": wvs[hg], "w2": w2s[hg], "ec": ecm})
    return in_maps


def make_ec():
    """E selector: ec[h, j*128+p] = 1 iff head-of-partition-p-in-tile-j == h."""
    ecm = np.zeros((8, CH), dtype=np.float32)
    for j in range(4):
        ecm[2 * j, j * P:j * P + 64] = 1.0
        ecm[2 * j + 1, j * P + 64:(j + 1) * P] = 1.0
    return ecm


def kernel(x, W_qkv, W_out, b_out):
    from concourse.bass_utils import run_bass_kernel_spmd

    nc = _get_nc(T_FULL)
    in_maps = make_in_maps(x, W_qkv, W_out, b_out)
    res = run_bass_kernel_spmd(nc, in_maps, core_ids=list(range(N_CORES))).results
    bo = np.asarray(b_out, dtype=np.float32)
    out = np.empty((B, T_FULL, DIM), dtype=np.float32)
    for b in range(B):
        out[b] = (res[2 * b]["y"].astype(np.float32)
                  + res[2 * b + 1]["y"].astype(np.float32) + bo)
    return out


# revision 3
# speedup vs baseline: 1.1502x; 1.0144x over previous
"""Linear attention kernel for 8 Trainium2 NeuronCores — fp8 DoubleRow version.

Sharding: core = 2*b + hg  (b in 0..3 batches, hg in 0..1 head-groups of 8 heads).
Fully data-parallel — no collectives; host sums the two head-group partials per
batch plus bias.

fp8 (float8_e4m3: max finite 240) is used where error cancels or is verified
negligible: q/k projections (DoubleRow, K=256/instr), qs and Ms for the y
matmul (DoubleRow). v projection, kv, z, den stay bf16 — v's error propagates
linearly to the output and dominates the budget otherwise.

Scales (powers of 2):
  x8    = fp8(16 x)          |x|<=5.42 -> <=87
  Wq8/Wk8 = fp8(1024 W)      |W|<=1/32 -> <=32     proj psum = 2^14 (x @ W)
  qt    = bf16(16 (elu+1))
  k_sb  = bf16(elu+1), v_sb = bf16(x @ Wv)
  dps   = 2^4 q.z ; rf = 2^-4 dps + 1e-6 ; rT = bf16(1/rf)
  bcp   = Es(2^14) @ rT = 2^14 r
  qs    = fp8(qt * bcp) = fp8(2^18 q r)  <= 2^18/min_z ~ 65
  Ms8   = fp8(2 * kvt @ W2)              maxM ~ 55 -> ~110
  y     = psum(qs @ Ms8) * 2^-19 -> bf16

elu identity (3 ops instead of 4):
  s*(elu(t)+1) = min(exp(t + ln s), s) + relu(s*t)
ACT exp (never overflows: |t|<=~6), ACT relu, then fused
scalar_tensor_tensor((e min s) add r) on DVE. GPSIMD is never used for bulk
elementwise work: it is ~10x slower than DVE/ACT and triggers heavy
power-throttling of the whole core.
"""

import sys

sys.path.insert(0, "/opt/trn_rl_repo")

import numpy as np

import concourse.bass as bass
import concourse.mybir as mybir
import concourse.tile as tile
from concourse import bacc

F32 = mybir.dt.float32
BF16 = mybir.dt.bfloat16
FP8 = mybir.dt.float8e4
AF = mybir.ActivationFunctionType
ALU = mybir.AluOpType
DR = mybir.MatmulPerfMode.DoubleRow

DIM = 1024      # model dim (contraction for projections)
CH = 512        # per-core channels (8 heads x 64)
P = 128

N_CORES = 8
B, T_FULL = 4, 4096

LN16 = float(np.log(16.0))
MS_SCALE = 2.0


def build_nc(T=T_FULL):
    NTB = T // 512          # 512-token blocks
    nc = bacc.Bacc(None, target_bir_lowering=False, debug=False)

    # xT c-major fp8(16x); xV c-major bf16 (v path needs full precision).
    # w1 fp8 [DIM, 2CH]: q | k (1024-scaled). wv bf16 [DIM, CH].
    xT = nc.declare_dram_parameter("xT", [DIM, T], FP8, isOutput=False)
    xV = nc.declare_dram_parameter("xV", [DIM, T], BF16, isOutput=False)
    w1 = nc.declare_dram_parameter("w1", [DIM, 2 * CH], FP8, isOutput=False)
    wv = nc.declare_dram_parameter("wv", [DIM, CH], BF16, isOutput=False)
    w2 = nc.declare_dram_parameter("w2", [CH, DIM], BF16, isOutput=False)
    ec = nc.declare_dram_parameter("ec", [P, CH], BF16, isOutput=False)
    y = nc.declare_dram_parameter("y", [T, DIM], BF16, isOutput=True)

    with tile.TileContext(nc) as tc:
        with tc.tile_pool(name="persist", bufs=1) as pp:
            # ---- constants / persistent tiles ----
            ones_col = pp.tile([P, 1], BF16, name="ones_col", tag="ones_col")
            nc.vector.memset(ones_col[:, :], 1.0)
            bias_q = pp.tile([P, 1], F32, name="bias_q", tag="bias_q")
            nc.vector.memset(bias_q[:, :], LN16)

            # w1 tiles: [P, 2, 2CH] fp8, ct2 = pair of 128-row k-blocks.
            # Loaded first: the single usable DMA queue (sync) serializes all
            # transfers, so only what the first q matmuls need goes ahead of
            # the ib0 x tiles; wv/w2/ec stream in behind them (phase_a).
            w1t = []
            for ct2 in range(4):
                t_ = pp.tile([P, 2, 2 * CH], FP8, name=f"w1_{ct2}", tag=f"w1_{ct2}")
                w1t.append(t_)
            wvts = []
            for ct in range(8):
                t_ = pp.tile([P, CH], BF16, name=f"wv_{ct}", tag=f"wv_{ct}")
                wvts.append(t_)

            # qt: bf16 c-major q, jp-paired layout [P, 2, T]
            qt = [
                pp.tile([P, 2, T], BF16, name=f"qt_{jp}", tag=f"qt_{jp}")
                for jp in range(2)
            ]

            kvt = [
                pp.tile([P, P], BF16, name=f"kvt_{j}", tag=f"kvt_{j}")
                for j in range(4)
            ]
            zt = pp.tile([P, 4], BF16, name="zt", tag="zt")

            w2t = []
            for j in range(4):
                t_ = pp.tile([P, DIM], BF16, name=f"w2_{j}", tag=f"w2_{j}")
                w2t.append(t_)
            ec_sb = pp.tile([P, CH], BF16, name="ec_sb", tag="ec_sb")

            rTbanks = [
                pp.tile([P, 512], BF16, name=f"rT_{g4}", tag=f"rT_{g4}")
                for g4 in range(NTB // 4)
            ]
            Zb = []
            for j in range(4):
                zb = pp.tile([P, 8], BF16, name=f"Zb_{j}", tag=f"Zb_{j}")
                Zb.append(zb)

            phase_a(nc, tc, pp, T, NTB, xT, xV, w1, w1t, wvts, qt, kvt, zt,
                    ones_col, bias_q, wv, w2, (ec, ec_sb), w2t, Zb, rTbanks)
            phase_b(nc, tc, pp, T, NTB, w2t, ec_sb, y, qt, kvt, zt, Zb, rTbanks)

    nc.compile()
    return nc


def emit_q(nc, pa, proj_ps, w1t, xt, qt, bias_q, ib, tsl, relu_dve=False):
    # q projection (c-major, fp8 DR) with elu+1 -> bf16 qt
    for j in range(4):
        jp, ji = j // 2, j % 2
        qps = proj_ps.tile([P, 512], F32, name=f"qps_{ib}_{j}", tag="proj")
        for ct2 in range(4):
            nc.tensor.matmul(
                qps[:, :],
                w1t[ct2][:, :, j * P:(j + 1) * P],
                xt[ct2][:, :, :],
                start=(ct2 == 0),
                stop=(ct2 == 3),
                perf_mode=DR,
            )
        e_ = pa.tile([P, 512], BF16, name=f"qe_{ib}_{j}", tag="elu_e")
        r_ = pa.tile([P, 512], BF16, name=f"qr_{ib}_{j}", tag="elu_r")
        nc.scalar.activation(e_[:, :], qps[:, :], AF.Exp,
                             bias=bias_q[:, :], scale=float(2.0 ** -14))
        if relu_dve:
            nc.vector.tensor_scalar(
                out=r_[:, :], in0=qps[:, :],
                scalar1=float(2.0 ** -10), scalar2=0.0,
                op0=ALU.mult, op1=ALU.max,
            )
        else:
            nc.scalar.activation(r_[:, :], qps[:, :], AF.Relu,
                                 scale=float(2.0 ** -10))
        nc.vector.scalar_tensor_tensor(
            out=qt[jp][:, ji, tsl], in0=e_[:, :], scalar=16.0,
            in1=r_[:, :], op0=ALU.min, op1=ALU.add,
        )


def phase_a(nc, tc, pp, T, NTB, xT, xV, w1, w1t, wvts, qt, kvt, zt,
            ones_col, bias_q, wv, w2, ec, w2t, Zb, rTbanks):
    deferred_q = []
    with (
        tc.tile_pool(name="phA_sb", bufs=3) as pa,
        tc.tile_pool(name="kv_sb", bufs=4) as kvp,
        tc.tile_pool(name="xload", bufs=8) as xp,
        tc.tile_pool(name="xvload", bufs=16) as xvp,
        tc.tile_pool(name="proj_ps", bufs=6, space="PSUM") as proj_ps,
        tc.tile_pool(name="hold_ps", bufs=1, space="PSUM") as hold_ps,
    ):
        # PSUM accumulators held across all of phase A (one bank each).
        kvps = hold_ps.tile([P, 4 * P], F32, name="kvps", tag="kvps")
        zps = hold_ps.tile([P, 4], F32, name="zps", tag="zps")
        nc.vector.memset(kvps[:, :], 0.0)
        nc.vector.memset(zps[:, :], 0.0)

        for ib in range(NTB):
            tsl = slice(ib * 512, (ib + 1) * 512)
            xt, xv = [], []
            for ct2 in range(4):
                t_ = xp.tile([P, 2, 512], FP8, name=f"xt_{ib}_{ct2}", tag="xt")
                for i in range(2):
                    r0 = ct2 * 256 + i * 128
                    if ib == 0:
                        nc.sync.dma_start(out=w1t[ct2][:, i, :],
                                          in_=w1[r0:r0 + P, :])
                    nc.sync.dma_start(out=t_[:, i, :], in_=xT[r0:r0 + P, tsl])
                xt.append(t_)
            for ct in range(8):
                t_ = xvp.tile([P, 512], BF16, name=f"xv_{ib}_{ct}", tag="xv")
                nc.gpsimd.dma_start(out=t_[:, :], in_=xV[ct * P:(ct + 1) * P, tsl])
                xv.append(t_)
            if ib == 0:
                # behind ib0's x tiles: v weights (needed in ~5us), then the
                # phase-B constants
                for ct in range(8):
                    nc.sync.dma_start(out=wvts[ct][:, :],
                                      in_=wv[ct * P:(ct + 1) * P, :])
                for j in range(4):
                    nc.sync.dma_start(out=w2t[j][:, :],
                                      in_=w2[j * P:(j + 1) * P, :])
                nc.sync.dma_start(out=ec[1][:, :], in_=ec[0][:, :])

            if ib < NTB - 1:
                emit_q(nc, pa, proj_ps, w1t, xt, qt, bias_q, ib, tsl)
            else:
                deferred_q.append((ib, tsl, xt))

            # ---- k (fp8 DR), v (bf16) per 128-token block ----
            for t in range(4):
                tok = slice(t * P, (t + 1) * P)
                kps = proj_ps.tile([P, 512], F32, name=f"kps_{ib}_{t}", tag="proj")
                for ct2 in range(4):
                    nc.tensor.matmul(
                        kps[:, :],
                        xt[ct2][:, :, tok],
                        w1t[ct2][:, :, CH:2 * CH],
                        start=(ct2 == 0),
                        stop=(ct2 == 3),
                        perf_mode=DR,
                    )
                ke = pa.tile([P, 512], BF16, name=f"ke_{ib}_{t}", tag="elu_e")
                kr = pa.tile([P, 512], BF16, name=f"kr_{ib}_{t}", tag="elu_r")
                k_sb = kvp.tile([P, 512], BF16, name=f"k_{ib}_{t}", tag="k_sb")
                nc.scalar.activation(ke[:, :], kps[:, :], AF.Exp,
                                     bias=0.0, scale=float(2.0 ** -14))
                nc.vector.tensor_scalar(
                    out=kr[:, :], in0=kps[:, :],
                    scalar1=float(2.0 ** -14), scalar2=0.0,
                    op0=ALU.mult, op1=ALU.max,
                )
                nc.vector.scalar_tensor_tensor(
                    out=k_sb[:, :], in0=ke[:, :], scalar=1.0,
                    in1=kr[:, :], op0=ALU.min, op1=ALU.add,
                )

                vps = proj_ps.tile([P, 512], F32, name=f"vps_{ib}_{t}", tag="proj")
                for ct in range(8):
                    nc.tensor.matmul(
                        vps[:, :],
                        xv[ct][:, tok],
                        wvts[ct][:, :],
                        start=(ct == 0),
                        stop=(ct == 7),
                    )
                v_sb = kvp.tile([P, 512], BF16, name=f"v_{ib}_{t}", tag="v_sb")
                nc.scalar.activation(v_sb[:, :], vps[:, :], AF.Copy)

                last = (ib == NTB - 1 and t == 3)
                # kvT[j] += v_pair^T k_pair   [128, 128] per head-pair, and
                # z[csl] += k_pair^T ones  [128, 1]: the 1-row z matmul's
                # 128-row weight load hides under the kv matmul's stream.
                for j in range(4):
                    csl = slice(j * P, (j + 1) * P)
                    nc.tensor.matmul(
                        kvps[:, csl], v_sb[:, csl], k_sb[:, csl],
                        start=False, stop=(last and j == 3),
                        skip_group_check=True,
                    )
                    nc.tensor.matmul(
                        zps[:, j:j + 1], k_sb[:, csl], ones_col[:, :],
                        start=False, stop=(last and j == 3),
                        skip_group_check=True,
                    )

        # z eviction + Zb build FIRST on DVE/queue so phase-B den can start
        # the moment the deferred-q matmuls drain
        nc.vector.tensor_copy(zt[:, :], zps[:, :])
        for j in range(4):
            nc.vector.memset(Zb[j][:, :], 0.0)
            nc.sync.dma_start(
                out=Zb[j][0:64, 2 * j:2 * j + 1],
                in_=zt[0:64, j:j + 1],
            )
            nc.sync.dma_start(
                out=Zb[j][64:128, 2 * j + 1:2 * j + 2],
                in_=zt[64:128, j:j + 1],
            )

        # deferred q projections: PE filler while DVE evicts kv/z
        for ib, tsl, xt in deferred_q:
            emit_q(nc, pa, proj_ps, w1t, xt, qt, bias_q, ib, tsl,
                   relu_dve=True)

        # ---- evict PSUM accumulators before releasing phase-A pools ----
        for j in range(4):
            nc.vector.memset(kvt[j][:, :], 0.0)
            nc.vector.tensor_copy(
                kvt[j][0:64, 0:64], kvps[0:64, j * P:j * P + 64]
            )
            nc.vector.tensor_copy(
                kvt[j][64:128, 64:128],
                kvps[64:128, j * P + 64:(j + 1) * P],
            )


def phase_b(nc, tc, pp, T, NTB, w2t, ec_sb, y, qt, kvt, zt, Zb, rTbanks):
    # Es copies at partition bases 0/32/64/96: bc for ib uses base 32*(ib%4)
    Es = [[ec_sb[32 * g:32 * g + 8, j * P:(j + 1) * P] for j in range(4)]
          for g in range(4)]

    with (
        tc.tile_pool(name="phB_sb", bufs=2) as pb,
        tc.tile_pool(name="qsc_pool", bufs=4) as qp,
        tc.tile_pool(name="phB_ps", bufs=2, space="PSUM") as bps,
        tc.tile_pool(name="y_ps", bufs=4, space="PSUM") as yps_pool,
    ):
        # den: 4 token-blocks per psum bank at partition bases 0/32/64/96 ->
        # ONE full-width rf + reciprocal serves 4 blocks
        for g4 in range(NTB // 4):
            dps = bps.tile([P, 512], F32, name=f"dps_{g4}", tag="d", bufs=2)
            # The 4 gi regions sit on DISJOINT partitions of the bank, so the
            # per-partition start=True zero-region cannot wipe siblings: no
            # explicit memset needed.
            for gi in range(4):
                ib = g4 * 4 + gi
                tsl = slice(ib * 512, (ib + 1) * 512)
                for j in range(4):
                    jp, ji = j // 2, j % 2
                    nc.tensor.matmul(
                        dps[32 * gi:32 * gi + 8, :], Zb[j][:, :],
                        qt[jp][:, ji, tsl],
                        start=(j == 0), stop=(j == 3),
                        skip_group_check=True,
                        tile_position=(0, 32 * gi),
                    )
            rf = pb.tile([P, 512], F32, name=f"rf_{g4}", tag="rf")
            nc.vector.tensor_scalar(
                out=rf[:, :], in0=dps[:, :], scalar1=float(2.0 ** -4),
                scalar2=1e-6, op0=ALU.mult, op1=ALU.add,
            )
            with nc.allow_low_precision(reason="r is O(1e-5); bf16 ok"):
                nc.vector.reciprocal(rTbanks[g4][:, :], rf[:, :])

        # Ms: fp8 jp-paired [P, 2, DIM], scaled by MS_SCALE
        Ms = []
        for jp in range(2):
            ms = pp.tile([P, 2, DIM], FP8, name=f"Ms_{jp}", tag=f"Ms_{jp}")
            for i in range(2):
                j = 2 * jp + i
                for h in range(2):
                    hsl = slice(h * 512, (h + 1) * 512)
                    mps = bps.tile([P, 512], F32, name=f"mps_{j}_{h}", tag="d",
                                   bufs=2)
                    nc.tensor.matmul(
                        mps[:, :], kvt[j][:, :], w2t[j][:, hsl],
                        start=True, stop=True,
                    )
                    if h == 0:
                        nc.vector.tensor_scalar_mul(ms[:, i, hsl], mps[:, :],
                                                    MS_SCALE)
                    else:
                        nc.scalar.activation(ms[:, i, hsl], mps[:, :], AF.Copy,
                                             scale=MS_SCALE)
            Ms.append(ms)

        qscs = []
        for ib in range(NTB):
            tsl = slice(ib * 512, (ib + 1) * 512)
            qsc = []
            for jp in range(2):
                qs = qp.tile([P, 2, 512], FP8, name=f"qsc_{ib}_{jp}",
                             tag="qsc", bufs=16)
                for i in range(2):
                    j = 2 * jp + i
                    g4, gi = ib // 4, ib % 4
                    bcp = bps.tile([P, 512], F32, name=f"bcp_{ib}_{j}", tag="bc")
                    nc.tensor.matmul(
                        bcp[:, :], Es[gi][j][:, :],
                        rTbanks[g4][32 * gi:32 * gi + 8, :],
                        start=True, stop=True,
                        tile_position=(32 * gi, 0),
                    )
                    nc.vector.tensor_tensor(
                        out=qs[:, i, :], in0=qt[jp][:, i, tsl], in1=bcp[:, :],
                        op=ALU.mult,
                    )
                qsc.append(qs)
            qscs.append(qsc)

        YS = float(2.0 ** -18 / MS_SCALE)
        for ib in range(NTB):
            qsc = qscs[ib]
            for t in range(4):
                tok = slice(t * P, (t + 1) * P)
                row = (ib * 4 + t) * P
                y_sb = pb.tile(
                    [P, DIM], BF16, name=f"y_{ib}_{t}", tag="y_sb", bufs=8
                )
                for h in range(2):
                    hsl = slice(h * 512, (h + 1) * 512)
                    yp = yps_pool.tile(
                        [P, 512], F32, name=f"yps_{ib}_{t}_{h}", tag="y"
                    )
                    for jp in range(2):
                        nc.tensor.matmul(
                            yp[:, :], qsc[jp][:, :, tok], Ms[jp][:, :, hsl],
                            start=(jp == 0), stop=(jp == 1),
                            perf_mode=DR,
                        )
                    if h == 0:
                        nc.vector.tensor_scalar_mul(y_sb[:, hsl], yp[:, :], YS)
                    else:
                        nc.scalar.activation(y_sb[:, hsl], yp[:, :], AF.Copy,
                                             scale=YS)
                if True:
                    nc.sync.dma_start(out=y[row:row + P, :], in_=y_sb[:, :])


_NC_CACHE = {}


def _get_nc(T=T_FULL):
    if T not in _NC_CACHE:
        _NC_CACHE[T] = build_nc(T)
    return _NC_CACHE[T]


def make_in_maps(x, W_qkv, W_out, b_out):
    import ml_dtypes

    bf16 = ml_dtypes.bfloat16
    f8 = ml_dtypes.float8_e4m3
    x = np.asarray(x, dtype=np.float32)
    W_qkv = np.asarray(W_qkv, dtype=np.float32)
    W_out = np.asarray(W_out, dtype=np.float32).astype(bf16)

    xTs, xVs = [], []
    for b in range(B):
        xt = np.ascontiguousarray(x[b].T)
        xTs.append(np.ascontiguousarray((16.0 * xt).astype(f8)))
        xVs.append(np.ascontiguousarray(xt.astype(bf16)))
    w1s, wvs, w2s = [], [], []
    for hg in range(2):
        cs = slice(hg * CH, (hg + 1) * CH)
        Wq = W_qkv[:, cs]
        Wk = W_qkv[:, DIM + hg * CH:DIM + (hg + 1) * CH]
        Wv = W_qkv[:, 2 * DIM + hg * CH:2 * DIM + (hg + 1) * CH]
        Wq8 = (1024.0 * Wq).astype(f8)
        Wk8 = (1024.0 * Wk).astype(f8)
        w1s.append(np.ascontiguousarray(np.concatenate([Wq8, Wk8], axis=1)))
        wvs.append(np.ascontiguousarray(Wv.astype(bf16)))
        w2s.append(np.ascontiguousarray(W_out[cs, :]))
    ec1 = make_ec() * (2.0 ** 14)
    ecm = np.zeros((P, CH), dtype=np.float32)
    for g in range(4):
        ecm[32 * g:32 * g + 8, :] = ec1
    ecm = ecm.astype(bf16)

    in_maps = []
    for core in range(N_CORES):
        b, hg = core // 2, core % 2
        in_maps.append({"xT": xTs[b], "xV": xVs[b], "w1": w1s[hg],
                        "wv": wvs[hg], "w2": w2s[hg], "ec": ecm})
    return in_maps


def make_ec():
    """E selector: ec[h, j*128+p] = 1 iff head-of-partition-p-in-tile-j == h."""
    ecm = np.zeros((8, CH), dtype=np.float32)
    for j in range(4):
        ecm[2 * j, j * P:j * P + 64] = 1.0
        ecm[2 * j + 1, j * P + 64:(j + 1) * P] = 1.0
    return ecm


def kernel(x, W_qkv, W_out, b_out):
    from concourse.bass_utils import run_bass_kernel_spmd

    nc = _get_nc(T_FULL)
    in_maps = make_in_maps(x, W_qkv, W_out, b_out)
    res = run_bass_kernel_spmd(nc, in_maps, core_ids=list(range(N_CORES))).results
    bo = np.asarray(b_out, dtype=np.float32)
    out = np.empty((B, T_FULL, DIM), dtype=np.float32)
    for b in range(B):
        out[b] = (res[2 * b]["y"].astype(np.float32)
                  + res[2 * b + 1]["y"].astype(np.float32) + bo)
    return out


# revision 4
# speedup vs baseline: 1.1634x; 1.0115x over previous
"""Linear attention kernel for 8 Trainium2 NeuronCores — fp8 DoubleRow version.

Sharding: core = 2*b + hg  (b in 0..3 batches, hg in 0..1 head-groups of 8 heads).
Fully data-parallel — no collectives; host sums the two head-group partials per
batch plus bias.

fp8 (float8_e4m3: max finite 240) is used where error cancels or is verified
negligible: q/k projections (DoubleRow, K=256/instr), qs and Ms for the y
matmul (DoubleRow). v projection, kv, z, den stay bf16 — v's error propagates
linearly to the output and dominates the budget otherwise.

Scales (powers of 2):
  x8    = fp8(16 x)          |x|<=5.42 -> <=87
  Wq8/Wk8 = fp8(1024 W)      |W|<=1/32 -> <=32     proj psum = 2^14 (x @ W)
  qt    = bf16(16 (elu+1))
  k_sb  = bf16(elu+1), v_sb = bf16(x @ Wv)
  dps   = 2^4 q.z ; rf = 2^-4 dps + 1e-6 ; rT = bf16(1/rf)
  bcp   = Es(2^14) @ rT = 2^14 r
  qs    = fp8(qt * bcp) = fp8(2^18 q r)  <= 2^18/min_z ~ 65
  Ms8   = fp8(2 * kvt @ W2)              maxM ~ 55 -> ~110
  y     = psum(qs @ Ms8) * 2^-19 -> bf16

elu identity (3 ops instead of 4):
  s*(elu(t)+1) = min(exp(t + ln s), s) + relu(s*t)
ACT exp (never overflows: |t|<=~6), ACT relu, then fused
scalar_tensor_tensor((e min s) add r) on DVE. GPSIMD is never used for bulk
elementwise work: it is ~10x slower than DVE/ACT and triggers heavy
power-throttling of the whole core.
"""

import sys

sys.path.insert(0, "/opt/trn_rl_repo")

import numpy as np

import concourse.bass as bass
import concourse.mybir as mybir
import concourse.tile as tile
from concourse import bacc

F32 = mybir.dt.float32
BF16 = mybir.dt.bfloat16
FP8 = mybir.dt.float8e4
AF = mybir.ActivationFunctionType
ALU = mybir.AluOpType
DR = mybir.MatmulPerfMode.DoubleRow

DIM = 1024      # model dim (contraction for projections)
CH = 512        # per-core channels (8 heads x 64)
P = 128

N_CORES = 8
B, T_FULL = 4, 4096

LN16 = float(np.log(16.0))
MS_SCALE = 2.0


def build_nc(T=T_FULL):
    NTB = T // 512          # 512-token blocks
    nc = bacc.Bacc(None, target_bir_lowering=False, debug=False)

    # xT c-major fp8(16x); xV c-major bf16 (v path needs full precision).
    # w1 fp8 [DIM, 2CH]: q | k (1024-scaled). wv bf16 [DIM, CH].
    xT = nc.declare_dram_parameter("xT", [DIM, T], FP8, isOutput=False)
    xV = nc.declare_dram_parameter("xV", [DIM, T], BF16, isOutput=False)
    w1 = nc.declare_dram_parameter("w1", [DIM, 2 * CH], FP8, isOutput=False)
    wv = nc.declare_dram_parameter("wv", [DIM, CH], BF16, isOutput=False)
    w2 = nc.declare_dram_parameter("w2", [CH, DIM], BF16, isOutput=False)
    ec = nc.declare_dram_parameter("ec", [P, CH], BF16, isOutput=False)
    y = nc.declare_dram_parameter("y", [T, DIM], BF16, isOutput=True)

    with tile.TileContext(nc) as tc:
        with tc.tile_pool(name="persist", bufs=1) as pp:
            # ---- constants / persistent tiles ----
            ones_col = pp.tile([P, 1], BF16, name="ones_col", tag="ones_col")
            nc.vector.memset(ones_col[:, :], 1.0)
            bias_q = pp.tile([P, 1], F32, name="bias_q", tag="bias_q")
            nc.vector.memset(bias_q[:, :], LN16)

            # w1 tiles: [P, 2, 2CH] fp8, ct2 = pair of 128-row k-blocks.
            # Loaded first: the single usable DMA queue (sync) serializes all
            # transfers, so only what the first q matmuls need goes ahead of
            # the ib0 x tiles; wv/w2/ec stream in behind them (phase_a).
            w1t = []
            for ct2 in range(4):
                t_ = pp.tile([P, 2, 2 * CH], FP8, name=f"w1_{ct2}", tag=f"w1_{ct2}")
                w1t.append(t_)
            wvts = []
            for ct in range(8):
                t_ = pp.tile([P, CH], BF16, name=f"wv_{ct}", tag=f"wv_{ct}")
                wvts.append(t_)

            # qt: bf16 c-major q, jp-paired layout [P, 2, T]
            qt = [
                pp.tile([P, 2, T], BF16, name=f"qt_{jp}", tag=f"qt_{jp}")
                for jp in range(2)
            ]

            kvt = [
                pp.tile([P, P], BF16, name=f"kvt_{j}", tag=f"kvt_{j}")
                for j in range(4)
            ]
            zt = pp.tile([P, 4], BF16, name="zt", tag="zt")

            w2t = []
            for j in range(4):
                t_ = pp.tile([P, DIM], BF16, name=f"w2_{j}", tag=f"w2_{j}")
                w2t.append(t_)
            ec_sb = pp.tile([P, CH], BF16, name="ec_sb", tag="ec_sb")

            rTbanks = [
                pp.tile([P, 512], BF16, name=f"rT_{g4}", tag=f"rT_{g4}")
                for g4 in range(NTB // 4)
            ]
            Zb = []
            for j in range(4):
                zb = pp.tile([P, 8], BF16, name=f"Zb_{j}", tag=f"Zb_{j}")
                Zb.append(zb)

            phase_a(nc, tc, pp, T, NTB, xT, xV, w1, w1t, wvts, qt, kvt, zt,
                    ones_col, bias_q, wv, w2, (ec, ec_sb), w2t, Zb, rTbanks)
            phase_b(nc, tc, pp, T, NTB, w2t, ec_sb, y, qt, kvt, zt, Zb, rTbanks)

    nc.compile()
    return nc


def emit_q(nc, pa, proj_ps, w1t, xt, qt, bias_q, ib, tsl, relu_dve=False):
    # q projection (c-major, fp8 DR) with elu+1 -> bf16 qt
    for j in range(4):
        jp, ji = j // 2, j % 2
        qps = proj_ps.tile([P, 512], F32, name=f"qps_{ib}_{j}", tag="proj")
        for ct2 in range(4):
            nc.tensor.matmul(
                qps[:, :],
                w1t[ct2][:, :, j * P:(j + 1) * P],
                xt[ct2][:, :, :],
                start=(ct2 == 0),
                stop=(ct2 == 3),
                perf_mode=DR,
            )
        e_ = pa.tile([P, 512], BF16, name=f"qe_{ib}_{j}", tag="elu_e")
        r_ = pa.tile([P, 512], BF16, name=f"qr_{ib}_{j}", tag="elu_r")
        nc.scalar.activation(e_[:, :], qps[:, :], AF.Exp,
                             bias=bias_q[:, :], scale=float(2.0 ** -14))
        if relu_dve:
            nc.vector.tensor_scalar(
                out=r_[:, :], in0=qps[:, :],
                scalar1=float(2.0 ** -10), scalar2=0.0,
                op0=ALU.mult, op1=ALU.max,
            )
        else:
            nc.scalar.activation(r_[:, :], qps[:, :], AF.Relu,
                                 scale=float(2.0 ** -10))
        nc.vector.scalar_tensor_tensor(
            out=qt[jp][:, ji, tsl], in0=e_[:, :], scalar=16.0,
            in1=r_[:, :], op0=ALU.min, op1=ALU.add,
        )


def phase_a(nc, tc, pp, T, NTB, xT, xV, w1, w1t, wvts, qt, kvt, zt,
            ones_col, bias_q, wv, w2, ec, w2t, Zb, rTbanks):
    deferred_q = []
    with (
        tc.tile_pool(name="phA_sb", bufs=3) as pa,
        tc.tile_pool(name="kv_sb", bufs=4) as kvp,
        tc.tile_pool(name="xload", bufs=8) as xp,
        tc.tile_pool(name="xvload", bufs=16) as xvp,
        tc.tile_pool(name="proj_ps", bufs=6, space="PSUM") as proj_ps,
        tc.tile_pool(name="hold_ps", bufs=1, space="PSUM") as hold_ps,
    ):
        # PSUM accumulators held across all of phase A (one bank each).
        kvps = hold_ps.tile([P, 4 * P], F32, name="kvps", tag="kvps")
        zps = hold_ps.tile([P, 4], F32, name="zps", tag="zps")
        nc.vector.memset(kvps[:, :], 0.0)
        nc.vector.memset(zps[:, :], 0.0)
        pending_kv = (None, None)

        for ib in range(NTB):
            tsl = slice(ib * 512, (ib + 1) * 512)
            xt, xv = [], []
            for ct2 in range(4):
                t_ = xp.tile([P, 2, 512], FP8, name=f"xt_{ib}_{ct2}", tag="xt")
                for i in range(2):
                    r0 = ct2 * 256 + i * 128
                    if ib == 0:
                        nc.sync.dma_start(out=w1t[ct2][:, i, :],
                                          in_=w1[r0:r0 + P, :])
                    nc.sync.dma_start(out=t_[:, i, :], in_=xT[r0:r0 + P, tsl])
                xt.append(t_)
            for ct in range(8):
                t_ = xvp.tile([P, 512], BF16, name=f"xv_{ib}_{ct}", tag="xv")
                nc.gpsimd.dma_start(out=t_[:, :], in_=xV[ct * P:(ct + 1) * P, tsl])
                xv.append(t_)
            if ib == 0:
                # v weights + phase-B constants ride the SWDGE queue, parallel
                # with the sync queue's w1/x stream (wv lands ~12us, in time
                # for v(ib0))
                for ct in range(8):
                    nc.gpsimd.dma_start(out=wvts[ct][:, :],
                                        in_=wv[ct * P:(ct + 1) * P, :])
                for j in range(4):
                    nc.gpsimd.dma_start(out=w2t[j][:, :],
                                        in_=w2[j * P:(j + 1) * P, :])
                nc.gpsimd.dma_start(out=ec[1][:, :], in_=ec[0][:, :])

            if ib < NTB - 1:
                emit_q(nc, pa, proj_ps, w1t, xt, qt, bias_q, ib, tsl)
            else:
                deferred_q.append((ib, tsl, xt))

            # ---- k (fp8 DR), v (bf16) per 128-token block ----
            for t in range(4):
                tok = slice(t * P, (t + 1) * P)
                kps = proj_ps.tile([P, 512], F32, name=f"kps_{ib}_{t}", tag="proj")
                for ct2 in range(4):
                    nc.tensor.matmul(
                        kps[:, :],
                        xt[ct2][:, :, tok],
                        w1t[ct2][:, :, CH:2 * CH],
                        start=(ct2 == 0),
                        stop=(ct2 == 3),
                        perf_mode=DR,
                    )
                ke = pa.tile([P, 512], BF16, name=f"ke_{ib}_{t}", tag="elu_e")
                kr = pa.tile([P, 512], BF16, name=f"kr_{ib}_{t}", tag="elu_r")
                k_sb = kvp.tile([P, 512], BF16, name=f"k_{ib}_{t}", tag="k_sb")
                nc.scalar.activation(ke[:, :], kps[:, :], AF.Exp,
                                     bias=0.0, scale=float(2.0 ** -14))
                nc.vector.tensor_scalar(
                    out=kr[:, :], in0=kps[:, :],
                    scalar1=float(2.0 ** -14), scalar2=0.0,
                    op0=ALU.mult, op1=ALU.max,
                )
                nc.vector.scalar_tensor_tensor(
                    out=k_sb[:, :], in0=ke[:, :], scalar=1.0,
                    in1=kr[:, :], op0=ALU.min, op1=ALU.add,
                )

                # kv/z of the PREVIOUS token-block interleave between this
                # block's eight 512-row v matmuls so every 128-row kv/z
                # weight load hides under a big moving stream.
                vps = proj_ps.tile([P, 512], F32, name=f"vps_{ib}_{t}", tag="proj")
                for ct in range(8):
                    nc.tensor.matmul(
                        vps[:, :],
                        xv[ct][:, tok],
                        wvts[ct][:, :],
                        start=(ct == 0),
                        stop=(ct == 7),
                    )
                    if pending_kv[0] is not None:
                        pv, pk = pending_kv
                        j = ct % 4
                        csl = slice(j * P, (j + 1) * P)
                        if ct < 4:
                            nc.tensor.matmul(
                                kvps[:, csl], pv[:, csl], pk[:, csl],
                                start=False, stop=False,
                                skip_group_check=True,
                            )
                        else:
                            nc.tensor.matmul(
                                zps[:, j:j + 1], pk[:, csl], ones_col[:, :],
                                start=False, stop=False,
                                skip_group_check=True,
                            )
                v_sb = kvp.tile([P, 512], BF16, name=f"v_{ib}_{t}", tag="v_sb")
                nc.scalar.activation(v_sb[:, :], vps[:, :], AF.Copy)
                pending_kv = (v_sb, k_sb)

        # flush the last token-block's kv/z (closes both psum groups)
        pv, pk = pending_kv
        for j in range(4):
            csl = slice(j * P, (j + 1) * P)
            nc.tensor.matmul(
                kvps[:, csl], pv[:, csl], pk[:, csl],
                start=False, stop=(j == 3), skip_group_check=True,
            )
            nc.tensor.matmul(
                zps[:, j:j + 1], pk[:, csl], ones_col[:, :],
                start=False, stop=(j == 3), skip_group_check=True,
            )

        # z eviction + Zb build FIRST on DVE/queue so phase-B den can start
        # the moment the deferred-q matmuls drain
        nc.vector.tensor_copy(zt[:, :], zps[:, :])
        for j in range(4):
            nc.vector.memset(Zb[j][:, :], 0.0)
            nc.sync.dma_start(
                out=Zb[j][0:64, 2 * j:2 * j + 1],
                in_=zt[0:64, j:j + 1],
            )
            nc.sync.dma_start(
                out=Zb[j][64:128, 2 * j + 1:2 * j + 2],
                in_=zt[64:128, j:j + 1],
            )

        # deferred q projections: PE filler while DVE evicts kv/z
        for ib, tsl, xt in deferred_q:
            emit_q(nc, pa, proj_ps, w1t, xt, qt, bias_q, ib, tsl,
                   relu_dve=True)

        # ---- evict PSUM accumulators before releasing phase-A pools ----
        for j in range(4):
            nc.vector.memset(kvt[j][:, :], 0.0)
            nc.vector.tensor_copy(
                kvt[j][0:64, 0:64], kvps[0:64, j * P:j * P + 64]
            )
            nc.vector.tensor_copy(
                kvt[j][64:128, 64:128],
                kvps[64:128, j * P + 64:(j + 1) * P],
            )


def phase_b(nc, tc, pp, T, NTB, w2t, ec_sb, y, qt, kvt, zt, Zb, rTbanks):
    # Es copies at partition bases 0/32/64/96: bc for ib uses base 32*(ib%4)
    Es = [[ec_sb[32 * g:32 * g + 8, j * P:(j + 1) * P] for j in range(4)]
          for g in range(4)]

    with (
        tc.tile_pool(name="phB_sb", bufs=2) as pb,
        tc.tile_pool(name="qsc_pool", bufs=4) as qp,
        tc.tile_pool(name="phB_ps", bufs=2, space="PSUM") as bps,
        tc.tile_pool(name="y_ps", bufs=4, space="PSUM") as yps_pool,
    ):
        # den: 4 token-blocks per psum bank at partition bases 0/32/64/96 ->
        # ONE full-width rf + reciprocal serves 4 blocks
        for g4 in range(NTB // 4):
            dps = bps.tile([P, 512], F32, name=f"dps_{g4}", tag="d", bufs=2)
            # The 4 gi regions sit on DISJOINT partitions of the bank, so the
            # per-partition start=True zero-region cannot wipe siblings: no
            # explicit memset needed.
            for gi in range(4):
                ib = g4 * 4 + gi
                tsl = slice(ib * 512, (ib + 1) * 512)
                for j in range(4):
                    jp, ji = j // 2, j % 2
                    nc.tensor.matmul(
                        dps[32 * gi:32 * gi + 8, :], Zb[j][:, :],
                        qt[jp][:, ji, tsl],
                        start=(j == 0), stop=(j == 3),
                        skip_group_check=True,
                        tile_position=(0, 32 * gi),
                    )
            rf = pb.tile([P, 512], F32, name=f"rf_{g4}", tag="rf")
            nc.vector.tensor_scalar(
                out=rf[:, :], in0=dps[:, :], scalar1=float(2.0 ** -4),
                scalar2=1e-6, op0=ALU.mult, op1=ALU.add,
            )
            with nc.allow_low_precision(reason="r is O(1e-5); bf16 ok"):
                nc.vector.reciprocal(rTbanks[g4][:, :], rf[:, :])

        # Ms: fp8 jp-paired [P, 2, DIM], scaled by MS_SCALE
        Ms = []
        for jp in range(2):
            ms = pp.tile([P, 2, DIM], FP8, name=f"Ms_{jp}", tag=f"Ms_{jp}")
            for i in range(2):
                j = 2 * jp + i
                for h in range(2):
                    hsl = slice(h * 512, (h + 1) * 512)
                    mps = bps.tile([P, 512], F32, name=f"mps_{j}_{h}", tag="d",
                                   bufs=2)
                    nc.tensor.matmul(
                        mps[:, :], kvt[j][:, :], w2t[j][:, hsl],
                        start=True, stop=True,
                    )
                    if h == 0:
                        nc.vector.tensor_scalar_mul(ms[:, i, hsl], mps[:, :],
                                                    MS_SCALE)
                    else:
                        nc.scalar.activation(ms[:, i, hsl], mps[:, :], AF.Copy,
                                             scale=MS_SCALE)
            Ms.append(ms)

        qscs = []
        for ib in range(NTB):
            tsl = slice(ib * 512, (ib + 1) * 512)
            qsc = []
            for jp in range(2):
                qs = qp.tile([P, 2, 512], FP8, name=f"qsc_{ib}_{jp}",
                             tag="qsc", bufs=16)
                for i in range(2):
                    j = 2 * jp + i
                    g4, gi = ib // 4, ib % 4
                    bcp = bps.tile([P, 512], F32, name=f"bcp_{ib}_{j}", tag="bc")
                    nc.tensor.matmul(
                        bcp[:, :], Es[gi][j][:, :],
                        rTbanks[g4][32 * gi:32 * gi + 8, :],
                        start=True, stop=True,
                        tile_position=(32 * gi, 0),
                    )
                    nc.vector.tensor_tensor(
                        out=qs[:, i, :], in0=qt[jp][:, i, tsl], in1=bcp[:, :],
                        op=ALU.mult,
                    )
                qsc.append(qs)
            qscs.append(qsc)

        YS = float(2.0 ** -18 / MS_SCALE)
        for ib in range(NTB):
            qsc = qscs[ib]
            for t in range(4):
                tok = slice(t * P, (t + 1) * P)
                row = (ib * 4 + t) * P
                y_sb = pb.tile(
                    [P, DIM], BF16, name=f"y_{ib}_{t}", tag="y_sb", bufs=8
                )
                for h in range(2):
                    hsl = slice(h * 512, (h + 1) * 512)
                    yp = yps_pool.tile(
                        [P, 512], F32, name=f"yps_{ib}_{t}_{h}", tag="y"
                    )
                    for jp in range(2):
                        nc.tensor.matmul(
                            yp[:, :], qsc[jp][:, :, tok], Ms[jp][:, :, hsl],
                            start=(jp == 0), stop=(jp == 1),
                            perf_mode=DR,
                        )
                    if h == 0:
                        nc.vector.tensor_scalar_mul(y_sb[:, hsl], yp[:, :], YS)
                    else:
                        nc.scalar.activation(y_sb[:, hsl], yp[:, :], AF.Copy,
                                             scale=YS)
                if True:
                    nc.sync.dma_start(out=y[row:row + P, :], in_=y_sb[:, :])


_NC_CACHE = {}


def _get_nc(T=T_FULL):
    if T not in _NC_CACHE:
        _NC_CACHE[T] = build_nc(T)
    return _NC_CACHE[T]


def make_in_maps(x, W_qkv, W_out, b_out):
    import ml_dtypes

    bf16 = ml_dtypes.bfloat16
    f8 = ml_dtypes.float8_e4m3
    x = np.asarray(x, dtype=np.float32)
    W_qkv = np.asarray(W_qkv, dtype=np.float32)
    W_out = np.asarray(W_out, dtype=np.float32).astype(bf16)

    xTs, xVs = [], []
    for b in range(B):
        xt = np.ascontiguousarray(x[b].T)
        xTs.append(np.ascontiguousarray((16.0 * xt).astype(f8)))
        xVs.append(np.ascontiguousarray(xt.astype(bf16)))
    w1s, wvs, w2s = [], [], []
    for hg in range(2):
        cs = slice(hg * CH, (hg + 1) * CH)
        Wq = W_qkv[:, cs]
        Wk = W_qkv[:, DIM + hg * CH:DIM + (hg + 1) * CH]
        Wv = W_qkv[:, 2 * DIM + hg * CH:2 * DIM + (hg + 1) * CH]
        Wq8 = (1024.0 * Wq).astype(f8)
        Wk8 = (1024.0 * Wk).astype(f8)
        w1s.append(np.ascontiguousarray(np.concatenate([Wq8, Wk8], axis=1)))
        wvs.append(np.ascontiguousarray(Wv.astype(bf16)))
        w2s.append(np.ascontiguousarray(W_out[cs, :]))
    ec1 = make_ec() * (2.0 ** 14)
    ecm = np.zeros((P, CH), dtype=np.float32)
    for g in range(4):
        ecm[32 * g:32 * g + 8, :] = ec1
    ecm = ecm.astype(bf16)

    in_maps = []
    for core in range(N_CORES):
        b, hg = core // 2, core % 2
        in_maps.append({"xT": xTs[b], "xV": xVs[b], "w1": w1s[hg],
                        "wv": wvs[hg], "w2": w2s[hg], "ec": ecm})
    return in_maps


def make_ec():
    """E selector: ec[h, j*128+p] = 1 iff head-of-partition-p-in-tile-j == h."""
    ecm = np.zeros((8, CH), dtype=np.float32)
    for j in range(4):
        ecm[2 * j, j * P:j * P + 64] = 1.0
        ecm[2 * j + 1, j * P + 64:(j + 1) * P] = 1.0
    return ecm


def kernel(x, W_qkv, W_out, b_out):
    from concourse.bass_utils import run_bass_kernel_spmd

    nc = _get_nc(T_FULL)
    in_maps = make_in_maps(x, W_qkv, W_out, b_out)
    res = run_bass_kernel_spmd(nc, in_maps, core_ids=list(range(N_CORES))).results
    bo = np.asarray(b_out, dtype=np.float32)
    out = np.empty((B, T_FULL, DIM), dtype=np.float32)
    for b in range(B):
        out[b] = (res[2 * b]["y"].astype(np.float32)
                  + res[2 * b + 1]["y"].astype(np.float32) + bo)
    return out
